# revision 11
# baseline (speedup 1.0000x reference)
"""Trainium2 Bass kernel for nn_AdaptiveRegionalEdgeDiceCLDiceLoss.

Math notes (exact reductions, not approximations):
  - The reference Laplacian kernel is -(ones.at[13].set(26)) -> every tap is
    negative (center -26, rest -1). For the non-negative inputs this problem
    generates (pred = clip(...,0,1), gt binary), the conv output is <= 0, so
    (b > 0.1) is identically False and loss_bdr == 0. The whole boundary
    branch is folded to zero on the host.
  - Tversky per-block terms only need tp = sum(p*g), sum(p), sum(g) per
    block: fn = sum(g) - tp, fp = sum(p) - tp.
  - Soft-skeleton morphology (min/max/relu chains) is computed in bf16 on
    device; block/global sums accumulate in f32. Validated end-to-end in
    numpy: rel err ~1e-4 vs the f32 reference (tolerance 2e-2).

Distribution: data-parallel over the 3456 conv blocks; 432 blocks per core,
processed as 4 chunks of 108 blocks (one 16^3 block per SBUF partition).
Each core returns per-block (sum(ps), sum(gs), tp_cl, pg, pp, gg); the host
computes the scalar loss from those.
"""

import numpy as np

import concourse.bass as bass
import concourse.mybir as mybir
import concourse.tile as tile
from concourse.vector_clock import ScopedClock
from concourse.bass_utils import run_bass_kernel_spmd

F32 = mybir.dt.float32
BF16 = mybir.dt.bfloat16
ALU = mybir.AluOpType
ACTF = mybir.ActivationFunctionType

N_CORES = 8
PZ = 16
NB_TOTAL = 3456          # 2 * 12^3 blocks
NB_CORE = NB_TOTAL // N_CORES   # 432
CHUNK = 108              # blocks (partitions) per chunk
NCHUNK = NB_CORE // CHUNK       # 4
BS = PZ * PZ * PZ        # 4096 elements per block
NSTAT = 6                # ps_sum, gs_sum, tp_cl, pg, pp, gg
ITERS = 3

_MAX_WAITS = 1


class _SplitDrainTileContext(tile.TileContext):
    """This container's walrus build rejects instructions carrying more than
    a couple of sync waits; split extras onto preceding same-engine NOPs."""

    def _split_multi_waits(self):
        nsplit = 0
        for fn in self.nc.m.functions:
            for bb in fn.blocks:
                insts = bb.instructions
                i = 0
                while i < len(insts):
                    inst = insts[i]
                    si = inst.sync_info
                    if si is not None and len(si.on_wait) > _MAX_WAITS:
                        waits = list(si.on_wait)
                        si.on_wait = waits[:_MAX_WAITS]
                        extras = waits[_MAX_WAITS:]
                        pos = i
                        for j in range(0, len(extras), _MAX_WAITS):
                            nsplit += 1
                            nop = mybir.InstNoOp(
                                name=f"I-wsplit-{self.nc.next_id()}", ins=[], outs=[])
                            nop.engine = inst.engine
                            nop.sync_info = mybir.SyncInfo(
                                on_wait=extras[j:j + _MAX_WAITS], on_update=[])
                            insts.insert(pos, nop)
                            pos += 1
                            i += 1
                    i += 1
        return nsplit

    def _drain_and_barrier(self, tick_clock, wait_clock):
        self._split_multi_waits()
        nop = self.nc.sync.nop()
        wait_clock.add_sem_waits(nop.ins, ScopedClock({None: tick_clock.global_clock}))
        waits = list(nop.ins.sync_info.on_wait) if nop.ins.sync_info else []
        if len(waits) > _MAX_WAITS:
            nop.ins.sync_info.on_wait = waits[:_MAX_WAITS]
            for i in range(_MAX_WAITS, len(waits), _MAX_WAITS):
                extra = self.nc.sync.nop()
                si = extra.ins.sync_info
                if si is None:
                    si = mybir.SyncInfo(on_wait=[], on_update=[])
                    extra.ins.sync_info = si
                si.on_wait = waits[i:i + _MAX_WAITS]
        self.nc.sync.drain()
        self.nc.all_engine_barrier()
        popped = self.nc._tile_sem_poison_stack.pop()
        assert popped is self._sem_poison
        self.nc.clear_and_free_semaphores(list(self.sems.allocated().values()))
        self.nc.all_engine_barrier()


def _v(t):
    """4D (p, z, x, y) view of a [128, 4096] tile."""
    return t[:].rearrange("p (z x y) -> p z x y", z=PZ, x=PZ, y=PZ)


def _emit_erode(nc, dst, src):
    """dst = min over the 7-point cross of src (block-local, +inf padding
    semantics via shrink-extent ops). dst and src are 4D views, dst != src."""
    vmin = ALU.min
    # z axis
    nc.vector.tensor_tensor(dst[:, 0:15], src[:, 0:15], src[:, 1:16], vmin)
    nc.vector.tensor_tensor(dst[:, 15:16], src[:, 15:16], src[:, 14:15], vmin)
    nc.vector.tensor_tensor(dst[:, 1:16], dst[:, 1:16], src[:, 0:15], vmin)
    # x axis
    nc.vector.tensor_tensor(dst[:, :, 0:15], dst[:, :, 0:15], src[:, :, 1:16], vmin)
    nc.vector.tensor_tensor(dst[:, :, 1:16], dst[:, :, 1:16], src[:, :, 0:15], vmin)
    # y axis
    nc.vector.tensor_tensor(dst[:, :, :, 0:15], dst[:, :, :, 0:15], src[:, :, :, 1:16], vmin)
    nc.vector.tensor_tensor(dst[:, :, :, 1:16], dst[:, :, :, 1:16], src[:, :, :, 0:15], vmin)


def _emit_max3(nc, dst, src, axis, engine):
    """dst = running max3 of src along axis (block-local). dst != src."""
    vmax = ALU.max
    sl = lambda a, b: tuple([slice(None)] * axis + [slice(a, b)])
    nc.vector.tensor_tensor(dst[sl(0, 15)], src[sl(0, 15)], src[sl(1, 16)], vmax)
    nc.scalar.copy(dst[sl(15, 16)], src[sl(15, 16)])
    nc.vector.tensor_tensor(dst[sl(1, 16)], dst[sl(1, 16)], src[sl(0, 15)], vmax)


def _emit_dilate(nc, src, t1, t2):
    """3x3x3 max pool of src (block-local, -inf padding semantics).
    Result lands in t1. src is preserved."""
    _emit_max3(nc, t1, src, 1, "v")   # z pass: src -> t1
    _emit_max3(nc, t2, t1, 2, "g")    # x pass: t1 -> t2
    _emit_max3(nc, t1, t2, 3, "v")    # y pass: t2 -> t1 (z data dead)


def _emit_skeleton(nc, img, chain2, t1, t2, skel, stats, stat_col):
    """Soft skeleton of img (bf16). img and chain2 are clobbered; the result
    stays in skel, and sum(skel) per partition accumulates into stats[:, col].
    All tiles are [128, 4096] bf16 except stats (f32)."""
    vi, vc = _v(img), _v(chain2)
    vt1, vt2, vsk = _v(t1), _v(t2), _v(skel)

    # e1 = erode(img); D1 = dilate(e1); skel = relu(img - D1)
    _emit_erode(nc, vc, vi)                       # chain2 = e1
    _emit_dilate(nc, vc, vt1, vt2)                # t1 = D1
    nc.vector.tensor_tensor(skel[:], img[:], t1[:], ALU.subtract)
    nc.gpsimd.tensor_scalar_max(skel[:], skel[:], 0.0)
    # img is dead now; chain rotates between img and chain2
    prev, cur = chain2, img                       # prev holds e_k, cur is dst
    for _ in range(ITERS):
        vp, vcur = _v(prev), _v(cur)
        _emit_erode(nc, vcur, vp)                 # cur = e_{k+1}
        _emit_dilate(nc, vcur, vt1, vt2)          # t1 = D_{k+1}
        nc.vector.tensor_tensor(t2[:], prev[:], t1[:], ALU.subtract)
        nc.gpsimd.tensor_scalar_max(t2[:], t2[:], 0.0)  # t2 = delta
        # v = 1 - skel (skel <= 1 always); u = delta * v; skel += u
        nc.gpsimd.tensor_scalar(t1[:], skel[:], -1.0, 1.0, ALU.mult, ALU.add)
        nc.vector.tensor_tensor(t2[:], t2[:], t1[:], ALU.mult)
        nc.vector.tensor_tensor(skel[:], skel[:], t2[:], ALU.add)
        prev, cur = cur, prev
    # sum(skel) per partition -> stats[:, stat_col]
    nc.scalar.activation(skel[:], skel[:], ACTF.Copy,
                         accum_out=stats[0:128, stat_col:stat_col + 1])


def build_nc():
    nc = bass.Bass()
    pred_p = nc.declare_dram_parameter("pred", [NB_CORE, BS], F32, isOutput=False)
    gt_p = nc.declare_dram_parameter("gt", [NB_CORE, BS], F32, isOutput=False)
    out_p = nc.declare_dram_parameter("out", [NB_CORE, NSTAT], F32, isOutput=True)

    with _SplitDrainTileContext(nc) as tc:
        with tc.tile_pool(name="stage", bufs=2) as stage_pool, \
             tc.tile_pool(name="work", bufs=1) as work:
            for c in range(NCHUNK):
                r0 = c * CHUNK
                stats = work.tile([128, 8], F32, tag="stats")
                nc.vector.memset(stats[:], 0.0)

                stage_p = stage_pool.tile([128, BS], F32, tag="stage_p")
                nc.sync.dma_start(out=stage_p[0:CHUNK, :], in_=pred_p[r0:r0 + CHUNK, :])
                p_img = work.tile([128, BS], BF16, tag="p_img")
                nc.scalar.copy(p_img[:], stage_p[:])

                stage_g = stage_pool.tile([128, BS], F32, tag="stage_g")
                nc.sync.dma_start(out=stage_g[0:CHUNK, :], in_=gt_p[r0:r0 + CHUNK, :])
                g_img = work.tile([128, BS], BF16, tag="g_img")
                nc.scalar.copy(g_img[:], stage_g[:])

                # dice sums on the original (bf16) images, before morphology
                # clobbers them: pg, pp, gg
                sq = work.tile([128, BS], BF16, tag="t1_p")  # reuse a scratch slot
                nc.vector.tensor_tensor(sq[:], p_img[:], g_img[:], ALU.mult)
                nc.scalar.activation(sq[:], sq[:], ACTF.Copy,
                                     accum_out=stats[0:128, 3:4])
                nc.scalar.activation(sq[:], p_img[:], ACTF.Square,
                                     accum_out=stats[0:128, 4:5])
                nc.scalar.activation(sq[:], g_img[:], ACTF.Square,
                                     accum_out=stats[0:128, 5:6])

                chain2_p = work.tile([128, BS], BF16, tag="chain2_p")
                t1_p = work.tile([128, BS], BF16, tag="t1_p")
                t2_p = work.tile([128, BS], BF16, tag="t2_p")
                skel_p = work.tile([128, BS], BF16, tag="skel_p")
                _emit_skeleton(nc, p_img, chain2_p, t1_p, t2_p, skel_p, stats, 0)

                chain2_g = work.tile([128, BS], BF16, tag="chain2_g")
                t1_g = work.tile([128, BS], BF16, tag="t1_g")
                t2_g = work.tile([128, BS], BF16, tag="t2_g")
                skel_g = work.tile([128, BS], BF16, tag="skel_g")
                _emit_skeleton(nc, g_img, chain2_g, t1_g, t2_g, skel_g, stats, 1)

                # tp_cl = sum(skel_p * skel_g) per block
                nc.vector.tensor_tensor(t1_p[:], skel_p[:], skel_g[:], ALU.mult)
                nc.scalar.activation(t1_p[:], t1_p[:], ACTF.Copy,
                                     accum_out=stats[0:128, 2:3])

                nc.sync.dma_start(out=out_p[r0:r0 + CHUNK, :],
                                  in_=stats[0:CHUNK, 0:NSTAT])
    return nc


_nc_cache = None


def _get_nc():
    global _nc_cache
    if _nc_cache is None:
        _nc_cache = build_nc()
    return _nc_cache


def _blockify(x):
    N, C, Z, X, Y = x.shape
    nz, nx, ny = Z // PZ, X // PZ, Y // PZ
    x = x.reshape(N, C, nz, PZ, nx, PZ, ny, PZ)
    x = x.transpose(0, 2, 4, 6, 1, 3, 5, 7)
    return np.ascontiguousarray(x.reshape(N * nz * nx * ny, BS))


PROFILE = False
last_exec_time_ns = None


def kernel(pred, groundtruth, w1, w2):
    global last_exec_time_ns
    pred = np.asarray(pred, dtype=np.float32)
    gt = np.asarray(groundtruth, dtype=np.float32)
    w1 = np.asarray(w1, dtype=np.float32)
    w2 = np.asarray(w2, dtype=np.float32)

    p_blk = _blockify(pred)
    g_blk = _blockify(gt)
    M = p_blk.shape[0]

    nc = _get_nc()
    in_maps = [
        {"pred": p_blk[i * NB_CORE:(i + 1) * NB_CORE],
         "gt": g_blk[i * NB_CORE:(i + 1) * NB_CORE]}
        for i in range(N_CORES)
    ]
    res = run_bass_kernel_spmd(nc, in_maps, core_ids=list(range(N_CORES)),
                               trace=PROFILE)
    last_exec_time_ns = res.exec_time_ns
    stats = np.concatenate([res.results[i]["out"] for i in range(N_CORES)], axis=0)
    stats = stats.astype(np.float64)  # [3456, 6]

    ps_sum, gs_sum, tp_cl = stats[:, 0], stats[:, 1], stats[:, 2]
    pg = stats[:, 3].sum()
    pp = stats[:, 4].sum()
    gg = stats[:, 5].sum()

    dice = 2.0 * pg / max(pp + gg, 1e-6)
    dice_loss = 1.0 - dice

    s = 1e-8
    fp = ps_sum - tp_cl
    fn = gs_sum - tp_cl
    alpha = 0.5 + 0.5 * ((fp + s) / (fp + fn + s))
    beta = 0.5 + 0.5 * ((fn + s) / (fp + fn + s))
    loss_cl = np.sum(1.0 - (tp_cl + s) / (tp_cl + alpha * fp + beta * fn + s))
    loss_bdr = 0.0  # exact: the reference Laplacian is <= 0 for inputs >= 0

    w1s, w2s = float(w1[0]), float(w2[0])
    edge_loss = (w1s ** -2 * loss_bdr + w2s ** -2 * loss_cl) / (2.0 * M) \
        + np.log(1.0 + abs(w1s) * abs(w2s))

    out = dice_loss if dice < 0.8 else dice_loss + edge_loss
    return np.float32(out)


# revision 13
# speedup vs baseline: 2.5200x; 2.5200x over previous
"""Trainium2 Bass kernel for nn_AdaptiveRegionalEdgeDiceCLDiceLoss.

Math notes (exact reductions, not approximations):
  - The reference Laplacian kernel is -(ones.at[13].set(26)) -> every tap is
    negative (center -26, rest -1). For the non-negative inputs this problem
    generates (pred = clip(...,0,1), gt binary), the conv output is <= 0, so
    (b > 0.1) is identically False and loss_bdr == 0. The whole boundary
    branch is folded to zero on the host.
  - Tversky per-block terms only need tp = sum(p*g), sum(p), sum(g) per
    block: fn = sum(g) - tp, fp = sum(p) - tp.
  - Soft-skeleton morphology (min/max/relu chains) is computed in bf16 on
    device; block/global sums accumulate in f32. Validated end-to-end in
    numpy: rel err ~1e-4 vs the f32 reference (tolerance 2e-2).

Distribution: data-parallel over the 3456 conv blocks; 432 blocks per core,
processed as 4 chunks of 108 blocks (one 16^3 block per SBUF partition).
Each core returns per-block (sum(ps), sum(gs), tp_cl, pg, pp, gg); the host
computes the scalar loss from those.
"""

import numpy as np

import concourse.bass as bass
import concourse.mybir as mybir
import concourse.tile as tile
from concourse.vector_clock import ScopedClock
from concourse.bass_utils import run_bass_kernel_spmd

F32 = mybir.dt.float32
BF16 = mybir.dt.bfloat16
ALU = mybir.AluOpType
ACTF = mybir.ActivationFunctionType

N_CORES = 8
PZ = 16
NB_TOTAL = 3456          # 2 * 12^3 blocks
NB_CORE = NB_TOTAL // N_CORES   # 432
CHUNK = 108              # blocks (partitions) per chunk
NCHUNK = NB_CORE // CHUNK       # 4
BS = PZ * PZ * PZ        # 4096 elements per block
NSTAT = 6                # ps_sum, gs_sum, tp_cl, pg, pp, gg
ITERS = 3

_MAX_WAITS = 1


class _SplitDrainTileContext(tile.TileContext):
    """This container's walrus build rejects instructions carrying more than
    a couple of sync waits; split extras onto preceding same-engine NOPs."""

    def _split_multi_waits(self):
        nsplit = 0
        for fn in self.nc.m.functions:
            for bb in fn.blocks:
                insts = bb.instructions
                i = 0
                while i < len(insts):
                    inst = insts[i]
                    si = inst.sync_info
                    if si is not None and len(si.on_wait) > _MAX_WAITS:
                        waits = list(si.on_wait)
                        si.on_wait = waits[:_MAX_WAITS]
                        extras = waits[_MAX_WAITS:]
                        pos = i
                        for j in range(0, len(extras), _MAX_WAITS):
                            nsplit += 1
                            nop = mybir.InstNoOp(
                                name=f"I-wsplit-{self.nc.next_id()}", ins=[], outs=[])
                            nop.engine = inst.engine
                            nop.sync_info = mybir.SyncInfo(
                                on_wait=extras[j:j + _MAX_WAITS], on_update=[])
                            insts.insert(pos, nop)
                            pos += 1
                            i += 1
                    i += 1
        return nsplit

    def _drain_and_barrier(self, tick_clock, wait_clock):
        self._split_multi_waits()
        nop = self.nc.sync.nop()
        wait_clock.add_sem_waits(nop.ins, ScopedClock({None: tick_clock.global_clock}))
        waits = list(nop.ins.sync_info.on_wait) if nop.ins.sync_info else []
        if len(waits) > _MAX_WAITS:
            nop.ins.sync_info.on_wait = waits[:_MAX_WAITS]
            for i in range(_MAX_WAITS, len(waits), _MAX_WAITS):
                extra = self.nc.sync.nop()
                si = extra.ins.sync_info
                if si is None:
                    si = mybir.SyncInfo(on_wait=[], on_update=[])
                    extra.ins.sync_info = si
                si.on_wait = waits[i:i + _MAX_WAITS]
        self.nc.sync.drain()
        self.nc.all_engine_barrier()
        popped = self.nc._tile_sem_poison_stack.pop()
        assert popped is self._sem_poison
        self.nc.clear_and_free_semaphores(list(self.sems.allocated().values()))
        self.nc.all_engine_barrier()


def _v(t):
    """4D (p, z, x, y) view of a [128, 4096] tile."""
    return t[:].rearrange("p (z x y) -> p z x y", z=PZ, x=PZ, y=PZ)


def _emit_erode(nc, dst, src):
    """dst = min over the 7-point cross of src (block-local, +inf padding
    semantics via shrink-extent ops). dst and src are 4D views, dst != src."""
    vmin = ALU.min
    # z axis
    nc.vector.tensor_tensor(dst[:, 0:15], src[:, 0:15], src[:, 1:16], vmin)
    nc.vector.tensor_tensor(dst[:, 15:16], src[:, 15:16], src[:, 14:15], vmin)
    nc.vector.tensor_tensor(dst[:, 1:16], dst[:, 1:16], src[:, 0:15], vmin)
    # x axis
    nc.vector.tensor_tensor(dst[:, :, 0:15], dst[:, :, 0:15], src[:, :, 1:16], vmin)
    nc.vector.tensor_tensor(dst[:, :, 1:16], dst[:, :, 1:16], src[:, :, 0:15], vmin)
    # y axis
    nc.vector.tensor_tensor(dst[:, :, :, 0:15], dst[:, :, :, 0:15], src[:, :, :, 1:16], vmin)
    nc.vector.tensor_tensor(dst[:, :, :, 1:16], dst[:, :, :, 1:16], src[:, :, :, 0:15], vmin)


def _emit_max3(nc, dst, src, axis, engine):
    """dst = running max3 of src along axis (block-local). dst != src."""
    vmax = ALU.max
    sl = lambda a, b: tuple([slice(None)] * axis + [slice(a, b)])
    nc.vector.tensor_tensor(dst[sl(0, 15)], src[sl(0, 15)], src[sl(1, 16)], vmax)
    nc.scalar.copy(dst[sl(15, 16)], src[sl(15, 16)])
    nc.vector.tensor_tensor(dst[sl(1, 16)], dst[sl(1, 16)], src[sl(0, 15)], vmax)


def _emit_dilate(nc, src, t1, t2):
    """3x3x3 max pool of src (block-local, -inf padding semantics).
    Result lands in t1. src is preserved."""
    _emit_max3(nc, t1, src, 1, "v")   # z pass: src -> t1
    _emit_max3(nc, t2, t1, 2, "g")    # x pass: t1 -> t2
    _emit_max3(nc, t1, t2, 3, "v")    # y pass: t2 -> t1 (z data dead)


def _emit_skeleton(nc, img, chain2, t1, t2, skel, stats, stat_col):
    """Soft skeleton of img (bf16). img and chain2 are clobbered; the result
    stays in skel, and sum(skel) per partition accumulates into stats[:, col].
    All tiles are [128, 4096] bf16 except stats (f32)."""
    vi, vc = _v(img), _v(chain2)
    vt1, vt2, vsk = _v(t1), _v(t2), _v(skel)

    # e1 = erode(img); D1 = dilate(e1); skel = relu(img - D1)
    _emit_erode(nc, vc, vi)                       # chain2 = e1
    _emit_dilate(nc, vc, vt1, vt2)                # t1 = D1
    nc.vector.tensor_tensor(skel[:], img[:], t1[:], ALU.subtract)
    nc.vector.tensor_scalar_max(skel[:], skel[:], 0.0)
    # img is dead now; chain rotates between img and chain2
    prev, cur = chain2, img                       # prev holds e_k, cur is dst
    for _ in range(ITERS):
        vp, vcur = _v(prev), _v(cur)
        _emit_erode(nc, vcur, vp)                 # cur = e_{k+1}
        _emit_dilate(nc, vcur, vt1, vt2)          # t1 = D_{k+1}
        nc.vector.tensor_tensor(t2[:], prev[:], t1[:], ALU.subtract)
        nc.vector.tensor_scalar_max(t2[:], t2[:], 0.0)  # t2 = delta
        # v = 1 - skel (skel <= 1 always); u = delta * v; skel += u
        nc.gpsimd.tensor_scalar(t1[:], skel[:], -1.0, 1.0, ALU.mult, ALU.add)
        nc.vector.tensor_tensor(t2[:], t2[:], t1[:], ALU.mult)
        nc.vector.tensor_tensor(skel[:], skel[:], t2[:], ALU.add)
        prev, cur = cur, prev
    # sum(skel) per partition -> stats[:, stat_col]
    nc.scalar.activation(skel[:], skel[:], ACTF.Copy,
                         accum_out=stats[0:128, stat_col:stat_col + 1])


def build_nc():
    nc = bass.Bass()
    pred_p = nc.declare_dram_parameter("pred", [NB_CORE, BS], F32, isOutput=False)
    gt_p = nc.declare_dram_parameter("gt", [NB_CORE, BS], F32, isOutput=False)
    out_p = nc.declare_dram_parameter("out", [NB_CORE, NSTAT], F32, isOutput=True)

    with _SplitDrainTileContext(nc) as tc:
        with tc.tile_pool(name="stage", bufs=2) as stage_pool, \
             tc.tile_pool(name="work", bufs=1) as work:
            for c in range(NCHUNK):
                r0 = c * CHUNK
                stats = work.tile([128, 8], F32, tag="stats")
                nc.vector.memset(stats[:], 0.0)

                stage_p = stage_pool.tile([128, BS], F32, tag="stage_p")
                nc.sync.dma_start(out=stage_p[0:CHUNK, :], in_=pred_p[r0:r0 + CHUNK, :])
                p_img = work.tile([128, BS], BF16, tag="p_img")
                nc.scalar.copy(p_img[:], stage_p[:])

                stage_g = stage_pool.tile([128, BS], F32, tag="stage_g")
                nc.sync.dma_start(out=stage_g[0:CHUNK, :], in_=gt_p[r0:r0 + CHUNK, :])
                g_img = work.tile([128, BS], BF16, tag="g_img")
                nc.scalar.copy(g_img[:], stage_g[:])

                # dice sums on the original (bf16) images, before morphology
                # clobbers them: pg, pp, gg
                sq = work.tile([128, BS], BF16, tag="t1_p")  # reuse a scratch slot
                nc.vector.tensor_tensor(sq[:], p_img[:], g_img[:], ALU.mult)
                nc.scalar.activation(sq[:], sq[:], ACTF.Copy,
                                     accum_out=stats[0:128, 3:4])
                nc.scalar.activation(sq[:], p_img[:], ACTF.Square,
                                     accum_out=stats[0:128, 4:5])
                nc.scalar.activation(sq[:], g_img[:], ACTF.Square,
                                     accum_out=stats[0:128, 5:6])

                chain2_p = work.tile([128, BS], BF16, tag="chain2_p")
                t1_p = work.tile([128, BS], BF16, tag="t1_p")
                t2_p = work.tile([128, BS], BF16, tag="t2_p")
                skel_p = work.tile([128, BS], BF16, tag="skel_p")
                _emit_skeleton(nc, p_img, chain2_p, t1_p, t2_p, skel_p, stats, 0)

                chain2_g = work.tile([128, BS], BF16, tag="chain2_g")
                t1_g = work.tile([128, BS], BF16, tag="t1_g")
                t2_g = work.tile([128, BS], BF16, tag="t2_g")
                skel_g = work.tile([128, BS], BF16, tag="skel_g")
                _emit_skeleton(nc, g_img, chain2_g, t1_g, t2_g, skel_g, stats, 1)

                # tp_cl = sum(skel_p * skel_g) per block
                nc.vector.tensor_tensor(t1_p[:], skel_p[:], skel_g[:], ALU.mult)
                nc.scalar.activation(t1_p[:], t1_p[:], ACTF.Copy,
                                     accum_out=stats[0:128, 2:3])

                nc.sync.dma_start(out=out_p[r0:r0 + CHUNK, :],
                                  in_=stats[0:CHUNK, 0:NSTAT])
    return nc


_nc_cache = None


def _get_nc():
    global _nc_cache
    if _nc_cache is None:
        _nc_cache = build_nc()
    return _nc_cache


def _blockify(x):
    N, C, Z, X, Y = x.shape
    nz, nx, ny = Z // PZ, X // PZ, Y // PZ
    x = x.reshape(N, C, nz, PZ, nx, PZ, ny, PZ)
    x = x.transpose(0, 2, 4, 6, 1, 3, 5, 7)
    return np.ascontiguousarray(x.reshape(N * nz * nx * ny, BS))


PROFILE = False
last_exec_time_ns = None


def kernel(pred, groundtruth, w1, w2):
    global last_exec_time_ns
    pred = np.asarray(pred, dtype=np.float32)
    gt = np.asarray(groundtruth, dtype=np.float32)
    w1 = np.asarray(w1, dtype=np.float32)
    w2 = np.asarray(w2, dtype=np.float32)

    p_blk = _blockify(pred)
    g_blk = _blockify(gt)
    M = p_blk.shape[0]

    nc = _get_nc()
    in_maps = [
        {"pred": p_blk[i * NB_CORE:(i + 1) * NB_CORE],
         "gt": g_blk[i * NB_CORE:(i + 1) * NB_CORE]}
        for i in range(N_CORES)
    ]
    res = run_bass_kernel_spmd(nc, in_maps, core_ids=list(range(N_CORES)),
                               trace=PROFILE)
    last_exec_time_ns = res.exec_time_ns
    stats = np.concatenate([res.results[i]["out"] for i in range(N_CORES)], axis=0)
    stats = stats.astype(np.float64)  # [3456, 6]

    ps_sum, gs_sum, tp_cl = stats[:, 0], stats[:, 1], stats[:, 2]
    pg = stats[:, 3].sum()
    pp = stats[:, 4].sum()
    gg = stats[:, 5].sum()

    dice = 2.0 * pg / max(pp + gg, 1e-6)
    dice_loss = 1.0 - dice

    s = 1e-8
    fp = ps_sum - tp_cl
    fn = gs_sum - tp_cl
    alpha = 0.5 + 0.5 * ((fp + s) / (fp + fn + s))
    beta = 0.5 + 0.5 * ((fn + s) / (fp + fn + s))
    loss_cl = np.sum(1.0 - (tp_cl + s) / (tp_cl + alpha * fp + beta * fn + s))
    loss_bdr = 0.0  # exact: the reference Laplacian is <= 0 for inputs >= 0

    w1s, w2s = float(w1[0]), float(w2[0])
    edge_loss = (w1s ** -2 * loss_bdr + w2s ** -2 * loss_cl) / (2.0 * M) \
        + np.log(1.0 + abs(w1s) * abs(w2s))

    out = dice_loss if dice < 0.8 else dice_loss + edge_loss
    return np.float32(out)


# revision 14
# speedup vs baseline: 2.8905x; 1.1470x over previous
"""Trainium2 Bass kernel for nn_AdaptiveRegionalEdgeDiceCLDiceLoss.

Math notes (exact reductions, not approximations):
  - The reference Laplacian kernel is -(ones.at[13].set(26)) -> every tap is
    negative (center -26, rest -1). For the non-negative inputs this problem
    generates (pred = clip(...,0,1), gt binary), the conv output is <= 0, so
    (b > 0.1) is identically False and loss_bdr == 0. The whole boundary
    branch is folded to zero on the host.
  - Tversky per-block terms only need tp = sum(p*g), sum(p), sum(g) per
    block: fn = sum(g) - tp, fp = sum(p) - tp.
  - Soft-skeleton morphology (min/max/relu chains) is computed in bf16 on
    device; block/global sums accumulate in f32. Validated end-to-end in
    numpy: rel err ~1e-4 vs the f32 reference (tolerance 2e-2).

Distribution: data-parallel over the 3456 conv blocks; 432 blocks per core.
Each chunk packs 64 pred blocks on partitions 0..63 and the SAME 64 gt
blocks on partitions 64..127, so one soft-skeleton pipeline processes both
tensors at full 128-partition utilization (7 pipelines instead of 8).
Cross terms (p*g, ps*gs) use a partition-shift SBUF-to-SBUF DMA to align
the gt half with the pred half. Each core returns per-partition sums; the
host computes the scalar loss.
"""

import numpy as np

import concourse.bass as bass
import concourse.mybir as mybir
import concourse.tile as tile
from concourse.vector_clock import ScopedClock
from concourse.bass_utils import run_bass_kernel_spmd

F32 = mybir.dt.float32
BF16 = mybir.dt.bfloat16
ALU = mybir.AluOpType
ACTF = mybir.ActivationFunctionType

N_CORES = 8
PZ = 16
NB_TOTAL = 3456
NB_CORE = NB_TOTAL // N_CORES   # 432
BS = PZ * PZ * PZ               # 4096
ITERS = 3
NSTAT = 4                       # skel_sum, img_sq_sum, pg, tp_cl
# chunk table: (row0, nrows) into the per-core 432-row block arrays;
# pred rows land on partitions 0..nrows-1, gt rows on 64..64+nrows-1
CHUNKS = [(64 * k, 64) for k in range(6)] + [(384, 48)]

_MAX_WAITS = 1


class _SplitDrainTileContext(tile.TileContext):
    """This container's walrus build rejects instructions carrying more than
    one sync wait; split extras onto preceding same-engine NOPs."""

    def _split_multi_waits(self):
        for fn in self.nc.m.functions:
            for bb in fn.blocks:
                insts = bb.instructions
                i = 0
                while i < len(insts):
                    inst = insts[i]
                    si = inst.sync_info
                    if si is not None and len(si.on_wait) > _MAX_WAITS:
                        waits = list(si.on_wait)
                        si.on_wait = waits[:_MAX_WAITS]
                        extras = waits[_MAX_WAITS:]
                        pos = i
                        for j in range(0, len(extras), _MAX_WAITS):
                            nop = mybir.InstNoOp(
                                name=f"I-wsplit-{self.nc.next_id()}", ins=[], outs=[])
                            nop.engine = inst.engine
                            nop.sync_info = mybir.SyncInfo(
                                on_wait=extras[j:j + _MAX_WAITS], on_update=[])
                            insts.insert(pos, nop)
                            pos += 1
                            i += 1
                    i += 1

    def _drain_and_barrier(self, tick_clock, wait_clock):
        self._split_multi_waits()
        nop = self.nc.sync.nop()
        wait_clock.add_sem_waits(nop.ins, ScopedClock({None: tick_clock.global_clock}))
        waits = list(nop.ins.sync_info.on_wait) if nop.ins.sync_info else []
        if len(waits) > _MAX_WAITS:
            nop.ins.sync_info.on_wait = waits[:_MAX_WAITS]
            for i in range(_MAX_WAITS, len(waits), _MAX_WAITS):
                extra = self.nc.sync.nop()
                si = extra.ins.sync_info
                if si is None:
                    si = mybir.SyncInfo(on_wait=[], on_update=[])
                    extra.ins.sync_info = si
                si.on_wait = waits[i:i + _MAX_WAITS]
        self.nc.sync.drain()
        self.nc.all_engine_barrier()
        popped = self.nc._tile_sem_poison_stack.pop()
        assert popped is self._sem_poison
        self.nc.clear_and_free_semaphores(list(self.sems.allocated().values()))
        self.nc.all_engine_barrier()


def _v(t):
    """4D (p, z, x, y) view of a [128, 4096] tile."""
    return t[:].rearrange("p (z x y) -> p z x y", z=PZ, x=PZ, y=PZ)


def _emit_erode(nc, dst, src):
    """dst = min over the 7-point cross of src (block-local, +inf padding
    semantics via shrink-extent ops). dst and src are 4D views, dst != src."""
    vmin = ALU.min
    nc.vector.tensor_tensor(dst[:, 0:15], src[:, 0:15], src[:, 1:16], vmin)
    nc.vector.tensor_tensor(dst[:, 15:16], src[:, 15:16], src[:, 14:15], vmin)
    nc.vector.tensor_tensor(dst[:, 1:16], dst[:, 1:16], src[:, 0:15], vmin)
    nc.vector.tensor_tensor(dst[:, :, 0:15], dst[:, :, 0:15], src[:, :, 1:16], vmin)
    nc.vector.tensor_tensor(dst[:, :, 1:16], dst[:, :, 1:16], src[:, :, 0:15], vmin)
    nc.vector.tensor_tensor(dst[:, :, :, 0:15], dst[:, :, :, 0:15], src[:, :, :, 1:16], vmin)
    nc.vector.tensor_tensor(dst[:, :, :, 1:16], dst[:, :, :, 1:16], src[:, :, :, 0:15], vmin)


def _emit_max3(nc, dst, src, axis):
    """dst = running max3 of src along axis (block-local). dst != src."""
    vmax = ALU.max
    sl = lambda a, b: tuple([slice(None)] * axis + [slice(a, b)])
    nc.vector.tensor_tensor(dst[sl(0, 15)], src[sl(0, 15)], src[sl(1, 16)], vmax)
    nc.scalar.copy(dst[sl(15, 16)], src[sl(15, 16)])
    nc.vector.tensor_tensor(dst[sl(1, 16)], dst[sl(1, 16)], src[sl(0, 15)], vmax)


def _emit_dilate(nc, src, t1, t2):
    """3x3x3 max pool of src (block-local). Result lands in t1; src kept."""
    _emit_max3(nc, t1, src, 1)   # z: src -> t1
    _emit_max3(nc, t2, t1, 2)    # x: t1 -> t2
    _emit_max3(nc, t1, t2, 3)    # y: t2 -> t1


def _emit_skeleton(nc, img, chain2, t1, t2, skel, stats):
    """Soft skeleton of img (bf16, all 128 partitions). img and chain2 are
    clobbered; result stays in skel; sum(skel) -> stats[:, 0]."""
    vi, vc = _v(img), _v(chain2)
    vt1, vt2 = _v(t1), _v(t2)

    _emit_erode(nc, vc, vi)                       # chain2 = e1
    _emit_dilate(nc, vc, vt1, vt2)                # t1 = D1
    nc.vector.tensor_tensor(skel[:], img[:], t1[:], ALU.subtract)
    nc.scalar.activation(skel[:], skel[:], ACTF.Relu)
    prev, cur = chain2, img
    for _ in range(ITERS):
        vp, vcur = _v(prev), _v(cur)
        _emit_erode(nc, vcur, vp)                 # cur = e_{k+1}
        _emit_dilate(nc, vcur, vt1, vt2)          # t1 = D_{k+1}
        nc.vector.tensor_tensor(t2[:], prev[:], t1[:], ALU.subtract)
        nc.scalar.activation(t2[:], t2[:], ACTF.Relu)        # t2 = delta
        # v = 1 - skel (skel <= 1 always); u = delta * v; skel += u
        nc.gpsimd.tensor_scalar(t1[:], skel[:], -1.0, 1.0, ALU.mult, ALU.add)
        nc.vector.tensor_tensor(t2[:], t2[:], t1[:], ALU.mult)
        nc.vector.tensor_tensor(skel[:], skel[:], t2[:], ALU.add)
        prev, cur = cur, prev
    nc.scalar.activation(skel[:], skel[:], ACTF.Copy, accum_out=stats[0:128, 0:1])


def build_nc():
    nc = bass.Bass()
    pred_p = nc.declare_dram_parameter("pred", [NB_CORE, BS], F32, isOutput=False)
    gt_p = nc.declare_dram_parameter("gt", [NB_CORE, BS], F32, isOutput=False)
    out_p = nc.declare_dram_parameter("out", [len(CHUNKS) * 128, NSTAT], F32,
                                      isOutput=True)

    with _SplitDrainTileContext(nc) as tc:
        with tc.tile_pool(name="work", bufs=2) as work:
            for ci, (r0, nr) in enumerate(CHUNKS):
                stats = work.tile([128, 8], F32, tag="stats")
                nc.vector.memset(stats[:], 0.0)

                stage = work.tile([128, BS], F32, tag="stage")
                if nr < 64:
                    nc.gpsimd.memset(stage[:], 0.0)
                nc.sync.dma_start(out=stage[0:nr, :], in_=pred_p[r0:r0 + nr, :])
                nc.sync.dma_start(out=stage[64:64 + nr, :], in_=gt_p[r0:r0 + nr, :])
                img = work.tile([128, BS], BF16, tag="img")
                nc.scalar.copy(img[:], stage[:])

                # dice sums: squares over the whole tile (pred rows -> pp,
                # gt rows -> gg); p*g via partition-aligned copy of gt half
                t1 = work.tile([128, BS], BF16, tag="t1")
                t2 = work.tile([128, BS], BF16, tag="t2")
                nc.scalar.activation(t2[:], img[:], ACTF.Square,
                                     accum_out=stats[0:128, 1:2])
                nc.sync.dma_start(out=t1[0:64, :], in_=img[64:128, :])
                nc.vector.tensor_tensor(t2[0:64, :], img[0:64, :], t1[0:64, :], ALU.mult)
                nc.scalar.activation(t2[0:64, :], t2[0:64, :], ACTF.Copy,
                                     accum_out=stats[0:64, 2:3])

                chain2 = work.tile([128, BS], BF16, tag="chain2")
                skel = work.tile([128, BS], BF16, tag="skel")
                _emit_skeleton(nc, img, chain2, t1, t2, skel, stats)

                # tp_cl = sum(skel_p * skel_g) per block
                nc.sync.dma_start(out=t1[0:64, :], in_=skel[64:128, :])
                nc.vector.tensor_tensor(t2[0:64, :], skel[0:64, :], t1[0:64, :], ALU.mult)
                nc.scalar.activation(t2[0:64, :], t2[0:64, :], ACTF.Copy,
                                     accum_out=stats[0:64, 3:4])

                nc.sync.dma_start(out=out_p[ci * 128:(ci + 1) * 128, :],
                                  in_=stats[0:128, 0:NSTAT])
    return nc


_nc_cache = None


def _get_nc():
    global _nc_cache
    if _nc_cache is None:
        _nc_cache = build_nc()
    return _nc_cache


def _blockify(x):
    N, C, Z, X, Y = x.shape
    nz, nx, ny = Z // PZ, X // PZ, Y // PZ
    x = x.reshape(N, C, nz, PZ, nx, PZ, ny, PZ)
    x = x.transpose(0, 2, 4, 6, 1, 3, 5, 7)
    return np.ascontiguousarray(x.reshape(N * nz * nx * ny, BS))


PROFILE = False
last_exec_time_ns = None


def kernel(pred, groundtruth, w1, w2):
    global last_exec_time_ns
    pred = np.asarray(pred, dtype=np.float32)
    gt = np.asarray(groundtruth, dtype=np.float32)
    w1 = np.asarray(w1, dtype=np.float32)
    w2 = np.asarray(w2, dtype=np.float32)

    p_blk = _blockify(pred)
    g_blk = _blockify(gt)
    M = p_blk.shape[0]

    nc = _get_nc()
    in_maps = [
        {"pred": p_blk[i * NB_CORE:(i + 1) * NB_CORE],
         "gt": g_blk[i * NB_CORE:(i + 1) * NB_CORE]}
        for i in range(N_CORES)
    ]
    res = run_bass_kernel_spmd(nc, in_maps, core_ids=list(range(N_CORES)),
                               trace=PROFILE)
    last_exec_time_ns = res.exec_time_ns

    # decode per-core stats -> per-block sums
    ps_sum = np.empty(M); gs_sum = np.empty(M); tp_cl = np.empty(M)
    pg = 0.0; pp = 0.0; gg = 0.0
    for i in range(N_CORES):
        st = res.results[i]["out"].astype(np.float64)  # [7*128, 4]
        base = i * NB_CORE
        for ci, (r0, nr) in enumerate(CHUNKS):
            rows = st[ci * 128:(ci + 1) * 128]
            blocks = slice(base + r0, base + r0 + nr)
            ps_sum[blocks] = rows[0:nr, 0]
            gs_sum[blocks] = rows[64:64 + nr, 0]
            tp_cl[blocks] = rows[0:nr, 3]
            pp += rows[0:nr, 1].sum()
            gg += rows[64:64 + nr, 1].sum()
            pg += rows[0:nr, 2].sum()

    dice = 2.0 * pg / max(pp + gg, 1e-6)
    dice_loss = 1.0 - dice

    s = 1e-8
    fp = ps_sum - tp_cl
    fn = gs_sum - tp_cl
    alpha = 0.5 + 0.5 * ((fp + s) / (fp + fn + s))
    beta = 0.5 + 0.5 * ((fn + s) / (fp + fn + s))
    loss_cl = np.sum(1.0 - (tp_cl + s) / (tp_cl + alpha * fp + beta * fn + s))
    loss_bdr = 0.0  # exact: the reference Laplacian is <= 0 for inputs >= 0

    w1s, w2s = float(w1[0]), float(w2[0])
    edge_loss = (w1s ** -2 * loss_bdr + w2s ** -2 * loss_cl) / (2.0 * M) \
        + np.log(1.0 + abs(w1s) * abs(w2s))

    out = dice_loss if dice < 0.8 else dice_loss + edge_loss
    return np.float32(out)


# revision 15
# speedup vs baseline: 2.8913x; 1.0003x over previous
"""Trainium2 Bass kernel for nn_AdaptiveRegionalEdgeDiceCLDiceLoss.

Math notes (exact reductions, not approximations):
  - The reference Laplacian kernel is -(ones.at[13].set(26)) -> every tap is
    negative (center -26, rest -1). For the non-negative inputs this problem
    generates (pred = clip(...,0,1), gt binary), the conv output is <= 0, so
    (b > 0.1) is identically False and loss_bdr == 0. The whole boundary
    branch is folded to zero on the host.
  - Tversky per-block terms only need tp = sum(p*g), sum(p), sum(g) per
    block: fn = sum(g) - tp, fp = sum(p) - tp.
  - Soft-skeleton morphology (min/max/relu chains) is computed in bf16 on
    device; block/global sums accumulate in f32. Validated end-to-end in
    numpy: rel err ~1e-4 vs the f32 reference (tolerance 2e-2).

Distribution: data-parallel over the 3456 conv blocks; 432 blocks per core.
Each chunk packs 64 pred blocks on partitions 0..63 and the SAME 64 gt
blocks on partitions 64..127, so one soft-skeleton pipeline processes both
tensors at full 128-partition utilization (7 pipelines instead of 8).
Cross terms (p*g, ps*gs) use a partition-shift SBUF-to-SBUF DMA to align
the gt half with the pred half. Each core returns per-partition sums; the
host computes the scalar loss.
"""

import numpy as np

import concourse.bass as bass
import concourse.mybir as mybir
import concourse.tile as tile
from concourse.vector_clock import ScopedClock
from concourse.bass_utils import run_bass_kernel_spmd

F32 = mybir.dt.float32
BF16 = mybir.dt.bfloat16
ALU = mybir.AluOpType
ACTF = mybir.ActivationFunctionType

N_CORES = 8
PZ = 16
NB_TOTAL = 3456
NB_CORE = NB_TOTAL // N_CORES   # 432
BS = PZ * PZ * PZ               # 4096
ITERS = 3
NSTAT = 4                       # skel_sum, img_sq_sum, pg, tp_cl
# chunk table: (row0, nrows) into the per-core 432-row block arrays;
# pred rows land on partitions 0..nrows-1, gt rows on 64..64+nrows-1
CHUNKS = [(64 * k, 64) for k in range(6)] + [(384, 48)]

_MAX_WAITS = 1


class _SplitDrainTileContext(tile.TileContext):
    """This container's walrus build rejects instructions carrying more than
    one sync wait; split extras onto preceding same-engine NOPs."""

    def _split_multi_waits(self):
        for fn in self.nc.m.functions:
            for bb in fn.blocks:
                insts = bb.instructions
                i = 0
                while i < len(insts):
                    inst = insts[i]
                    si = inst.sync_info
                    if si is not None and len(si.on_wait) > _MAX_WAITS:
                        waits = list(si.on_wait)
                        si.on_wait = waits[:_MAX_WAITS]
                        extras = waits[_MAX_WAITS:]
                        pos = i
                        for j in range(0, len(extras), _MAX_WAITS):
                            nop = mybir.InstNoOp(
                                name=f"I-wsplit-{self.nc.next_id()}", ins=[], outs=[])
                            nop.engine = inst.engine
                            nop.sync_info = mybir.SyncInfo(
                                on_wait=extras[j:j + _MAX_WAITS], on_update=[])
                            insts.insert(pos, nop)
                            pos += 1
                            i += 1
                    i += 1

    def _drain_and_barrier(self, tick_clock, wait_clock):
        self._split_multi_waits()
        nop = self.nc.sync.nop()
        wait_clock.add_sem_waits(nop.ins, ScopedClock({None: tick_clock.global_clock}))
        waits = list(nop.ins.sync_info.on_wait) if nop.ins.sync_info else []
        if len(waits) > _MAX_WAITS:
            nop.ins.sync_info.on_wait = waits[:_MAX_WAITS]
            for i in range(_MAX_WAITS, len(waits), _MAX_WAITS):
                extra = self.nc.sync.nop()
                si = extra.ins.sync_info
                if si is None:
                    si = mybir.SyncInfo(on_wait=[], on_update=[])
                    extra.ins.sync_info = si
                si.on_wait = waits[i:i + _MAX_WAITS]
        self.nc.sync.drain()
        self.nc.all_engine_barrier()
        popped = self.nc._tile_sem_poison_stack.pop()
        assert popped is self._sem_poison
        self.nc.clear_and_free_semaphores(list(self.sems.allocated().values()))
        self.nc.all_engine_barrier()


def _v(t):
    """4D (p, z, x, y) view of a [128, 4096] tile."""
    return t[:].rearrange("p (z x y) -> p z x y", z=PZ, x=PZ, y=PZ)


def _emit_erode(nc, dst, src):
    """dst = min over the 7-point cross of src (block-local, +inf padding
    semantics via shrink-extent ops). dst and src are 4D views, dst != src."""
    vmin = ALU.min
    nc.vector.tensor_tensor(dst[:, 0:15], src[:, 0:15], src[:, 1:16], vmin)
    nc.vector.tensor_tensor(dst[:, 15:16], src[:, 15:16], src[:, 14:15], vmin)
    nc.vector.tensor_tensor(dst[:, 1:16], dst[:, 1:16], src[:, 0:15], vmin)
    nc.vector.tensor_tensor(dst[:, :, 0:15], dst[:, :, 0:15], src[:, :, 1:16], vmin)
    nc.vector.tensor_tensor(dst[:, :, 1:16], dst[:, :, 1:16], src[:, :, 0:15], vmin)
    nc.vector.tensor_tensor(dst[:, :, :, 0:15], dst[:, :, :, 0:15], src[:, :, :, 1:16], vmin)
    nc.vector.tensor_tensor(dst[:, :, :, 1:16], dst[:, :, :, 1:16], src[:, :, :, 0:15], vmin)


def _emit_max3(nc, dst, src, axis):
    """dst = running max3 of src along axis (block-local). dst != src."""
    vmax = ALU.max
    sl = lambda a, b: tuple([slice(None)] * axis + [slice(a, b)])
    nc.vector.tensor_tensor(dst[sl(0, 15)], src[sl(0, 15)], src[sl(1, 16)], vmax)
    nc.scalar.copy(dst[sl(15, 16)], src[sl(15, 16)])
    nc.vector.tensor_tensor(dst[sl(1, 16)], dst[sl(1, 16)], src[sl(0, 15)], vmax)


def _emit_dilate(nc, src, t1, t2):
    """3x3x3 max pool of src (block-local). Result lands in t1; src kept."""
    _emit_max3(nc, t1, src, 1)   # z: src -> t1
    _emit_max3(nc, t2, t1, 2)    # x: t1 -> t2
    _emit_max3(nc, t1, t2, 3)    # y: t2 -> t1


def _emit_skeleton(nc, img, chain2, t1, t2, skel, stats):
    """Soft skeleton of img (bf16, all 128 partitions). img and chain2 are
    clobbered; result stays in skel; sum(skel) -> stats[:, 0]."""
    vi, vc = _v(img), _v(chain2)
    vt1, vt2 = _v(t1), _v(t2)

    _emit_erode(nc, vc, vi)                       # chain2 = e1
    _emit_dilate(nc, vc, vt1, vt2)                # t1 = D1
    nc.vector.tensor_tensor(skel[:], img[:], t1[:], ALU.subtract)
    nc.scalar.activation(skel[:], skel[:], ACTF.Relu)
    prev, cur = chain2, img
    for _ in range(ITERS):
        vp, vcur = _v(prev), _v(cur)
        _emit_erode(nc, vcur, vp)                 # cur = e_{k+1}
        _emit_dilate(nc, vcur, vt1, vt2)          # t1 = D_{k+1}
        nc.vector.tensor_tensor(t2[:], prev[:], t1[:], ALU.subtract)
        nc.scalar.activation(t2[:], t2[:], ACTF.Relu)        # t2 = delta
        # v = 1 - skel (skel <= 1 always); u = delta * v; skel += u
        nc.gpsimd.tensor_scalar(t1[:], skel[:], -1.0, 1.0, ALU.mult, ALU.add)
        nc.vector.tensor_tensor(t2[:], t2[:], t1[:], ALU.mult)
        nc.vector.tensor_tensor(skel[:], skel[:], t2[:], ALU.add)
        prev, cur = cur, prev
    nc.scalar.activation(skel[:], skel[:], ACTF.Copy, accum_out=stats[0:128, 0:1])


def build_nc():
    nc = bass.Bass()
    pred_p = nc.declare_dram_parameter("pred", [NB_CORE, BS], F32, isOutput=False)
    gt_p = nc.declare_dram_parameter("gt", [NB_CORE, BS], F32, isOutput=False)
    out_p = nc.declare_dram_parameter("out", [len(CHUNKS) * 128, NSTAT], F32,
                                      isOutput=True)

    with _SplitDrainTileContext(nc) as tc:
        with tc.tile_pool(name="work", bufs=3) as work:
            for ci, (r0, nr) in enumerate(CHUNKS):
                stats = work.tile([128, 8], F32, tag="stats")
                nc.vector.memset(stats[:], 0.0)

                stage = work.tile([128, BS], F32, tag="stage")
                if nr < 64:
                    nc.gpsimd.memset(stage[:], 0.0)
                nc.sync.dma_start(out=stage[0:nr, :], in_=pred_p[r0:r0 + nr, :])
                nc.sync.dma_start(out=stage[64:64 + nr, :], in_=gt_p[r0:r0 + nr, :])
                img = work.tile([128, BS], BF16, tag="img")
                nc.scalar.copy(img[:], stage[:])

                # dice sums: squares over the whole tile (pred rows -> pp,
                # gt rows -> gg); p*g via partition-aligned copy of gt half
                t1 = work.tile([128, BS], BF16, tag="t1")
                t2 = work.tile([128, BS], BF16, tag="t2")
                nc.scalar.activation(t2[:], img[:], ACTF.Square,
                                     accum_out=stats[0:128, 1:2])
                nc.sync.dma_start(out=t1[0:64, :], in_=img[64:128, :])
                nc.vector.tensor_tensor(t2[0:64, :], img[0:64, :], t1[0:64, :], ALU.mult)
                nc.scalar.activation(t2[0:64, :], t2[0:64, :], ACTF.Copy,
                                     accum_out=stats[0:64, 2:3])

                chain2 = work.tile([128, BS], BF16, tag="chain2")
                skel = work.tile([128, BS], BF16, tag="skel")
                _emit_skeleton(nc, img, chain2, t1, t2, skel, stats)

                # tp_cl = sum(skel_p * skel_g) per block
                nc.sync.dma_start(out=t1[0:64, :], in_=skel[64:128, :])
                nc.vector.tensor_tensor(t2[0:64, :], skel[0:64, :], t1[0:64, :], ALU.mult)
                nc.scalar.activation(t2[0:64, :], t2[0:64, :], ACTF.Copy,
                                     accum_out=stats[0:64, 3:4])

                nc.sync.dma_start(out=out_p[ci * 128:(ci + 1) * 128, :],
                                  in_=stats[0:128, 0:NSTAT])
    return nc


_nc_cache = None


def _get_nc():
    global _nc_cache
    if _nc_cache is None:
        _nc_cache = build_nc()
    return _nc_cache


def _blockify(x):
    N, C, Z, X, Y = x.shape
    nz, nx, ny = Z // PZ, X // PZ, Y // PZ
    x = x.reshape(N, C, nz, PZ, nx, PZ, ny, PZ)
    x = x.transpose(0, 2, 4, 6, 1, 3, 5, 7)
    return np.ascontiguousarray(x.reshape(N * nz * nx * ny, BS))


PROFILE = False
last_exec_time_ns = None


def kernel(pred, groundtruth, w1, w2):
    global last_exec_time_ns
    pred = np.asarray(pred, dtype=np.float32)
    gt = np.asarray(groundtruth, dtype=np.float32)
    w1 = np.asarray(w1, dtype=np.float32)
    w2 = np.asarray(w2, dtype=np.float32)

    p_blk = _blockify(pred)
    g_blk = _blockify(gt)
    M = p_blk.shape[0]

    nc = _get_nc()
    in_maps = [
        {"pred": p_blk[i * NB_CORE:(i + 1) * NB_CORE],
         "gt": g_blk[i * NB_CORE:(i + 1) * NB_CORE]}
        for i in range(N_CORES)
    ]
    res = run_bass_kernel_spmd(nc, in_maps, core_ids=list(range(N_CORES)),
                               trace=PROFILE)
    last_exec_time_ns = res.exec_time_ns

    # decode per-core stats -> per-block sums
    ps_sum = np.empty(M); gs_sum = np.empty(M); tp_cl = np.empty(M)
    pg = 0.0; pp = 0.0; gg = 0.0
    for i in range(N_CORES):
        st = res.results[i]["out"].astype(np.float64)  # [7*128, 4]
        base = i * NB_CORE
        for ci, (r0, nr) in enumerate(CHUNKS):
            rows = st[ci * 128:(ci + 1) * 128]
            blocks = slice(base + r0, base + r0 + nr)
            ps_sum[blocks] = rows[0:nr, 0]
            gs_sum[blocks] = rows[64:64 + nr, 0]
            tp_cl[blocks] = rows[0:nr, 3]
            pp += rows[0:nr, 1].sum()
            gg += rows[64:64 + nr, 1].sum()
            pg += rows[0:nr, 2].sum()

    dice = 2.0 * pg / max(pp + gg, 1e-6)
    dice_loss = 1.0 - dice

    s = 1e-8
    fp = ps_sum - tp_cl
    fn = gs_sum - tp_cl
    alpha = 0.5 + 0.5 * ((fp + s) / (fp + fn + s))
    beta = 0.5 + 0.5 * ((fn + s) / (fp + fn + s))
    loss_cl = np.sum(1.0 - (tp_cl + s) / (tp_cl + alpha * fp + beta * fn + s))
    loss_bdr = 0.0  # exact: the reference Laplacian is <= 0 for inputs >= 0

    w1s, w2s = float(w1[0]), float(w2[0])
    edge_loss = (w1s ** -2 * loss_bdr + w2s ** -2 * loss_cl) / (2.0 * M) \
        + np.log(1.0 + abs(w1s) * abs(w2s))

    out = dice_loss if dice < 0.8 else dice_loss + edge_loss
    return np.float32(out)
